# revision 2
# baseline (speedup 1.0000x reference)
"""Multi-head attention Trainium2 kernel (B=2, S=4096, D=512, H=8).

Sharding: 8 cores = 2 batches x 4 sequence-chunks of the query. Each core
computes full attention (all 8 heads) for its 1024 query rows against the
full 4096-long key/value sequence of its batch, including the output
projection. Host combine is a pure concat.

Per-core dataflow (flash-attention style, scores never touch HBM):
  - inputs are host-transposed to [D, S] layout (f32, fed as float32r)
  - projections on PE:  kT[dk,S] = Wk.T-tiles.T @ xkT,  qT likewise,
    v'[S, h*65] natural layout with a ones-column per head (row-sum trick)
  - scoresT[k,q] = kT-slice.T @ qT-slice  (K=64 contraction, two heads
    row-packed in the PE array via base partitions 0/64)
  - ACT: exp on 3-bank PSUM groups [128,1536] -> bf16 SBUF (scale=1/8)
  - PV: attT'[65,q] += v'-slice.T @ expP  accumulated over 32 k-tiles in
    PSUM; row 64 is the softmax denominator
  - normalize: DVE reciprocal + GpSimd partition-broadcast + DVE multiply
    -> attn[c, q] in f32r
  - out projection: out[q, :] = attn-slices.T @ Wo.T-tiles
"""

import sys

sys.path.insert(0, "/opt/trn_rl_repo")
sys.path.insert(0, "/root/.axon_site/_ro/trn_rl_repo")

import numpy as np

B, S, D, H, DK = 2, 4096, 512, 8, 64
NCORES = 8
BSHARD = NCORES // B          # 4 sequence shards per batch
SQ = S // BSHARD              # 1024 query rows per core
NCH = SQ // 512               # 2 q-chunks of 512
NKT = S // 128                # 32 k-tiles
NPAIR = H // 2                # 4 head pairs
VW = DK + 1                   # 65: per-head v width incl. ones column

_cache: dict = {}


def _build():
    if "nc" in _cache:
        return _cache["nc"]

    import concourse.mybir as mybir
    import concourse.tile as tile
    from concourse import bacc
    from concourse.bass import ts

    F32R = mybir.dt.float32r
    F32 = mybir.dt.float32
    BF16 = mybir.dt.bfloat16
    EXP = mybir.ActivationFunctionType.Exp

    nc = bacc.Bacc("TRN2", target_bir_lowering=False, debug=False,
                   num_devices=NCORES)

    xq = nc.dram_tensor("xq_t", [D, SQ], F32R, kind="ExternalInput")
    xk = nc.dram_tensor("xk_t", [D, S], F32R, kind="ExternalInput")
    xv = nc.dram_tensor("xv_t", [D, S], F32R, kind="ExternalInput")
    wq = nc.dram_tensor("wq_t", [D, D], F32R, kind="ExternalInput")
    wk = nc.dram_tensor("wk_t", [D, D], F32R, kind="ExternalInput")
    wv = nc.dram_tensor("wv_t", [D, D], F32R, kind="ExternalInput")
    wo = nc.dram_tensor("wo_t", [D, D], F32R, kind="ExternalInput")
    out = nc.dram_tensor("out", [SQ, D], F32, kind="ExternalOutput")

    with tile.TileContext(nc) as tc:
        with (
            tc.tile_pool(name="kt", bufs=4) as kt_pool,
            tc.tile_pool(name="qt", bufs=4) as qt_pool,
            tc.tile_pool(name="vp", bufs=32) as vp_pool,
            tc.tile_pool(name="attn", bufs=4) as attn_pool,
            tc.tile_pool(name="wop", bufs=4) as wo_pool,
            tc.tile_pool(name="ep", bufs=3) as ep_pool,
            tc.tile_pool(name="sm", bufs=2) as sm_pool,
            tc.tile_pool(name="ob", bufs=2) as ob_pool,
        ):
            kt = [kt_pool.tile([128, S], F32R, tag="kt", name=f"kt{i}") for i in range(4)]
            qt = [qt_pool.tile([128, SQ], F32R, tag="qt", name=f"qt{i}") for i in range(4)]
            vp = [vp_pool.tile([128, H * VW], BF16, tag="vp", name=f"vp{i}")
                  for i in range(NKT)]
            attn = [attn_pool.tile([128, SQ], F32R, tag="attn", name=f"attn{i}")
                    for i in range(4)]
            wot = [wo_pool.tile([128, D], F32R, tag="wo", name=f"wot{i}") for i in range(4)]
            for d in range(4):
                nc.sync.dma_start(wot[d][:], wo[ts(d, 128), :])

            # ---------------- projections ----------------
            with (
                tc.tile_pool(name="wtmp", bufs=4) as w_pool,
                tc.tile_pool(name="xs", bufs=8) as xs_pool,
                tc.tile_pool(name="pp", bufs=4, space="PSUM") as pp_pool,
            ):
                # kT[dk, s] = sum_d Wk.T[d, dk] * xkT[d, s]
                wkt = [w_pool.tile([128, D], F32R, tag="w", name=f"w{i}") for i in range(4)]
                for d in range(4):
                    nc.sync.dma_start(wkt[d][:], wk[ts(d, 128), :])
                for ch in range(S // 512):
                    xst = [xs_pool.tile([128, 512], F32R, tag="xs", name=f"xs{i}")
                           for i in range(4)]
                    for d in range(4):
                        nc.sync.dma_start(
                            xst[d][:], xk[ts(d, 128), ts(ch, 512)])
                    for m in range(4):
                        ps = pp_pool.tile([128, 512], F32, tag="pp")
                        for d in range(4):
                            nc.tensor.matmul(
                                ps[:], wkt[d][:, ts(m, 128)], xst[d][:],
                                start=(d == 0), stop=(d == 3))
                        nc.scalar.copy(kt[m][:, ts(ch, 512)], ps[:])

                # qT[dk, q] likewise from the core's query slice
                wqt = [w_pool.tile([128, D], F32R, tag="w", name=f"w{i}") for i in range(4)]
                for d in range(4):
                    nc.sync.dma_start(wqt[d][:], wq[ts(d, 128), :])
                for ch in range(NCH):
                    xst = [xs_pool.tile([128, 512], F32R, tag="xs", name=f"xs{i}")
                           for i in range(4)]
                    for d in range(4):
                        nc.sync.dma_start(
                            xst[d][:], xq[ts(d, 128), ts(ch, 512)])
                    for m in range(4):
                        ps = pp_pool.tile([128, 512], F32, tag="pp")
                        for d in range(4):
                            nc.tensor.matmul(
                                ps[:], wqt[d][:, ts(m, 128)], xst[d][:],
                                start=(d == 0), stop=(d == 3))
                        nc.scalar.copy(qt[m][:, ts(ch, 512)], ps[:])

                # v'[s, h*65] = xvT-slices.T @ Wv.T, plus ones columns
                wvt = [w_pool.tile([128, D], F32R, tag="w", name=f"w{i}") for i in range(4)]
                for d in range(4):
                    nc.sync.dma_start(wvt[d][:], wv[ts(d, 128), :])
                for sc in range(S // 512):
                    xst = [xs_pool.tile([128, 512], F32R, tag="xs", name=f"xs{i}")
                           for i in range(4)]
                    for d in range(4):
                        nc.sync.dma_start(
                            xst[d][:], xv[ts(d, 128), ts(sc, 512)])
                    for st in range(4):
                        k_idx = sc * 4 + st
                        ps = pp_pool.tile([128, 512], F32, tag="pp")
                        for d in range(4):
                            nc.tensor.matmul(
                                ps[:], xst[d][:, ts(st, 128)], wvt[d][:],
                                start=(d == 0), stop=(d == 3))
                        v3 = vp[k_idx][:].rearrange(
                            "p (h c) -> p h c", c=VW)
                        nc.gpsimd.memset(v3[:, :, DK:VW], 1.0)
                        nc.vector.tensor_copy(
                            v3[:, :, 0:DK],
                            ps[:].rearrange("p (h c) -> p h c", c=DK))

            # ---------------- attention + output projection ----------------
            with (
                tc.tile_pool(name="sc", bufs=2, space="PSUM") as sc_pool,
                tc.tile_pool(name="acc", bufs=2, space="PSUM") as acc_pool,
            ):
                for ch in range(NCH):
                    for p in range(NPAIR):
                        att = [acc_pool.tile([VW, 512], F32, tag="acc", name=f"att{i}")
                               for i in range(2)]
                        # (head, ktile) sequence; heads of the pair adjacent
                        seq = [(2 * p + hh, k) for k in range(NKT)
                               for hh in range(2)]
                        for g in range(0, len(seq), 3):
                            items = seq[g:g + 3]
                            n = len(items)
                            scps = sc_pool.tile([128, 1536], F32, tag="sc")
                            for slot, (h, k) in enumerate(items):
                                off = (h % 2) * 64
                                nc.tensor.matmul(
                                    scps[:, ts(slot, 512)],
                                    kt[p][off:off + 64, ts(k, 128)],
                                    qt[p][off:off + 64, ts(ch, 512)],
                                    start=True, stop=True)
                            ep = ep_pool.tile([128, 1536], BF16, tag="ep")
                            nc.scalar.activation(
                                ep[:, 0:n * 512], scps[:, 0:n * 512],
                                EXP, scale=0.125)
                            for slot, (h, k) in enumerate(items):
                                nc.tensor.matmul(
                                    att[h % 2][:],
                                    vp[k][:, h * VW:(h + 1) * VW],
                                    ep[:, ts(slot, 512)],
                                    start=(k == 0), stop=(k == NKT - 1))
                        for hh in range(2):
                            rc = sm_pool.tile([1, 512], F32, tag="rc")
                            nc.vector.reciprocal(rc[:], att[hh][DK:VW, :])
                            rep = sm_pool.tile([64, 512], F32, tag="rep")
                            nc.gpsimd.partition_broadcast(rep[:], rc[:])
                            nc.vector.tensor_mul(
                                attn[p][hh * 64:(hh + 1) * 64, ts(ch, 512)],
                                att[hh][0:DK, :], rep[:])
                    # output projection for this chunk of 512 q rows
                    for sbi in range(4):
                        po = acc_pool.tile([128, 512], F32, tag="acc")
                        for ct in range(4):
                            nc.tensor.matmul(
                                po[:],
                                attn[ct][:, ch * 512 + sbi * 128:
                                         ch * 512 + (sbi + 1) * 128],
                                wot[ct][:],
                                start=(ct == 0), stop=(ct == 3))
                        oo = ob_pool.tile([128, 512], F32, tag="ob")
                        nc.vector.tensor_copy(oo[:], po[:])
                        nc.sync.dma_start(
                            out[ch * 512 + sbi * 128:
                                ch * 512 + (sbi + 1) * 128, :], oo[:])

    nc.compile()
    _cache["nc"] = nc
    return nc


def kernel(query, key, value, Wq, Wk, Wv, Wo, _trace=False, _trace_cores=None):
    from concourse.bass_utils import run_bass_kernel_spmd

    query = np.asarray(query, dtype=np.float32)
    key = np.asarray(key, dtype=np.float32)
    value = np.asarray(value, dtype=np.float32)
    w_maps = {
        "wq_t": np.ascontiguousarray(np.asarray(Wq, dtype=np.float32).T),
        "wk_t": np.ascontiguousarray(np.asarray(Wk, dtype=np.float32).T),
        "wv_t": np.ascontiguousarray(np.asarray(Wv, dtype=np.float32).T),
        "wo_t": np.ascontiguousarray(np.asarray(Wo, dtype=np.float32).T),
    }

    nc = _build()

    in_maps = []
    for c in range(NCORES):
        b, sh = divmod(c, BSHARD)
        qs = sh * SQ
        xq_t = np.ascontiguousarray(query[b].T[:, qs:qs + SQ])
        xk_t = np.ascontiguousarray(key[b].T)
        xv_t = np.ascontiguousarray(value[b].T)
        in_maps.append({"xq_t": xq_t, "xk_t": xk_t, "xv_t": xv_t, **w_maps})

    res = run_bass_kernel_spmd(
        nc, in_maps, core_ids=list(range(NCORES)),
        trace=_trace, trace_cores=_trace_cores)
    kernel.last_results = res

    full = np.empty((B, S, D), dtype=np.float32)
    for c in range(NCORES):
        b, sh = divmod(c, BSHARD)
        qs = sh * SQ
        full[b, qs:qs + SQ] = res.results[c]["out"]
    return full
